# revision 9
# baseline (speedup 1.0000x reference)
"""Trainium2 Bass kernel for nn_Encoder_61830349193463 (retrieval_knn).

v3 strategy (data-parallel over src rows, 8 NeuronCores):
  - Each core gets a 2048-row shard of src; anchors + weights replicated.
  - kNN sims via a SINGLE bf16 PE matmul pass:
      sim = sh.ah + (256 - 0.5*||a||^2)  [bias folded in as split-bf16
      rows of a K=4 ones matmul]
  - All 4 anchor chunks ([128, 8192] bf16 each) stay RESIDENT in SBUF, so
    the loop runs tile-major: for each 128-row tile, 4 quarter matmuls into
    PSUM with max8 + find_index8 scans DIRECTLY on PSUM, then the
    candidate post-processing (merge -> gather -> exact refine -> top-5
    mask -> mean) runs inline and pipelines against the next tile's
    matmuls/scans (keeps the PE warm; v2 serialized this after all sims).
  - Top-6 anchors gathered fp32; ranks 4..6 re-scored EXACTLY as
    d2 = sum((s-g)^2): DVE subtract + ScalarE Square-accumulate; best 2 of
    the refined 3 + unconditional top-3 = exact top-5 (mask via is_le
    against the 4th-largest of a padded 8-slot score row).
  - mean(top5) = sum_k w_k * G_k via 6 chained DVE stt ops (w in {0,1},
    the /K folded into W_dim host-side); transpose to feature-major
    deferred to phase B (PSUM fully owned by the sims in phase A).
  - Dense chain: feature-major, BN stats AllReduced (3 tiny collectives),
    all matmuls bf16.
"""

import numpy as np

import concourse.bacc as bacc
import concourse.bass as bass
import concourse.mybir as mybir
import concourse.tile as tile
from concourse.bass import IndirectOffsetOnAxis
from concourse.bass_utils import run_bass_kernel_spmd
from concourse.masks import make_identity
import ml_dtypes

F32 = mybir.dt.float32
BF16 = mybir.dt.bfloat16
U32 = mybir.dt.uint32
AF = mybir.ActivationFunctionType
OP = mybir.AluOpType
P = 128

# problem sizes (hardcoded per contract)
N_FULL, M, D, F = 16384, 8192, 512, 2048
N_CORES = 8
K = 5
EPS = 1e-5
CAND = 6            # gathered candidates; ranks 4..6 exactly re-scored


def build_kernel(ns=N_FULL // N_CORES, m=M, d=D, f=F, n_cores=N_CORES,
                 mc_free=512, q_div=4):
    """Build the SPMD Bass module. ns/m/d/f sizes are per-core."""
    DC = d // P          # contraction chunks of the d dim (4)
    FC = f // P          # chunks of the hidden dim (16)
    T = ns // P          # n-tiles per core (16)
    nbf = min(mc_free, ns)
    NB = ns // nbf       # n blocks of 512 for phase-B matmuls (4)
    MQ = m // q_div      # m-quarter size (2048)
    QC = MQ // mc_free   # 512-chunks per quarter (4)
    NQ = 8 * q_div       # merged candidate pool width (32)
    NTOT = float(ns * n_cores)

    nc = bacc.Bacc("TRN2", target_bir_lowering=False, debug=False,
                   num_devices=n_cores)

    def param(name, shape, dt=F32):
        return nc.declare_dram_parameter(name, list(shape), dt, isOutput=False)

    srcT_h = param("srcT_h", [d, ns], BF16)
    src_nat = param("src_nat", [ns, d], F32)
    anchT_h = param("anchT_h", [d, m], BF16)
    caug = param("caug", [4, m], BF16)             # rows: c_h, c_l, 0, 0
    anchor = param("anchor", [m, d], F32)          # natural, for the gather
    wdim = param("wdim", [d, d], BF16)             # pre-scaled by 1/K
    wfus = param("wfus", [2 * d, d], BF16)
    we1 = param("we1", [d, f], BF16)
    we2 = param("we2", [f, d], BF16)
    wd = param("wd", [d, d], BF16)
    bdim = param("bdim", [P, DC])
    bfus = param("bfus", [P, DC])
    be1 = param("be1", [P, FC])
    be2 = param("be2", [P, DC])
    bd = param("bd", [P, DC])
    g1 = param("g1", [P, DC]); bt1 = param("bt1", [P, DC])
    g2 = param("g2", [P, DC]); bt2 = param("bt2", [P, DC])
    gd = param("gd", [P, DC]); btd = param("btd", [P, DC])
    out = nc.declare_dram_parameter("out", [ns, d], F32, isOutput=True)

    # internal DRAM for the three BN-stat AllReduces
    cc_in = [nc.dram_tensor(f"cc{i}_in", [P, 2 * DC], F32) for i in range(3)]
    cc_space = "Shared" if n_cores > 4 else "Local"
    cc_out = [nc.dram_tensor(f"cc{i}_out", [P, 2 * DC], F32,
                             addr_space=cc_space) for i in range(3)]
    groups = [list(range(n_cores))]

    with tile.TileContext(nc) as tc:
        with (
            tc.tile_pool(name="persist", bufs=1) as pp,
            tc.tile_pool(name="wpool", bufs=1) as wp,
        ):
            ident = pp.tile([P, P], F32, name="ident")
            make_identity(nc, ident[:])
            ones4 = pp.tile([4, P], BF16, name="ones4")
            nc.vector.memset(ones4[:], 1.0)
            ones8 = pp.tile([P, 8], F32, name="ones8")
            nc.vector.memset(ones8[:], 1.0)

            # resident bf16 source (hi split only)
            sTh = []
            for c in range(DC):
                th = pp.tile([P, ns], BF16, tag=f"sTh{c}", name=f"sTh{c}")
                nc.sync.dma_start(out=th[:], in_=srcT_h[c * P:(c + 1) * P, :])
                sTh.append(th)

            # feature-major neighbour means, filled by DMA-transposes in A2
            neighT = [pp.tile([P, ns], BF16, tag=f"nT{c}", name=f"nT{c}")
                      for c in range(DC)]

            # ================= PHASE A: kNN =================
            anch_ctx = tc.tile_pool(name="anch", bufs=1)
            anp = anch_ctx.__enter__()
            aT = []
            for c in range(DC):
                at = anp.tile([P, m], BF16, tag=f"aT{c}", name=f"aT{c}")
                nc.sync.dma_start(out=at[:], in_=anchT_h[c * P:(c + 1) * P, :])
                aT.append(at)
            caugt = anp.tile([4, m], BF16, tag="caug", name="caug")
            nc.sync.dma_start(out=caugt[:], in_=caug[:, :])

            with (
                tc.tile_pool(name="dps", bufs=2, space="PSUM") as dps,
                tc.tile_pool(name="tops", bufs=3) as tops,
                tc.tile_pool(name="mrg", bufs=2) as mrg,
                tc.tile_pool(name="gat", bufs=2) as gat,
                tc.tile_pool(name="snp", bufs=2) as snp,
            ):
                for t in range(T):
                    n_sl = slice(t * P, (t + 1) * P)
                    sn = snp.tile([P, d], F32, tag="sn", name="sn")
                    nc.sync.dma_start(out=sn[:],
                                      in_=src_nat[t * P:(t + 1) * P, :])
                    vcand = mrg.tile([P, NQ], F32, tag="vcand", name="vcand")
                    icand = mrg.tile([P, NQ], F32, tag="icand", name="icand")
                    for q in range(q_div):
                        ps = dps.tile([P, MQ], F32, name="dps")
                        for c in range(DC):
                            for mc in range(QC):
                                m_sl = slice(q * MQ + mc * mc_free,
                                             q * MQ + (mc + 1) * mc_free)
                                p_sl = slice(mc * mc_free, (mc + 1) * mc_free)
                                nc.tensor.matmul(ps[:, p_sl], sTh[c][:, n_sl],
                                                 aT[c][:, m_sl],
                                                 start=(c == 0), stop=False)
                        for mc in range(QC):
                            m_sl = slice(q * MQ + mc * mc_free,
                                         q * MQ + (mc + 1) * mc_free)
                            p_sl = slice(mc * mc_free, (mc + 1) * mc_free)
                            nc.tensor.matmul(ps[:, p_sl], ones4[:],
                                             caugt[:, m_sl],
                                             start=False, stop=True)
                        v8 = vcand[:, q * 8:(q + 1) * 8]
                        nc.vector.max(out=v8, in_=ps[:])
                        i8 = tops.tile([P, 8], U32, tag="i8", name="i8")
                        nc.vector.max_index(out=i8[:], in_max=v8,
                                            in_values=ps[:])
                        nc.vector.tensor_scalar(
                            out=icand[:, q * 8:(q + 1) * 8], in0=i8[:],
                            scalar1=float(q * MQ), scalar2=None, op0=OP.add)

                    # ---- merge quarters to global top-6 ----
                    g8 = mrg.tile([P, 8], F32, tag="g8", name="g8")
                    nc.vector.max(out=g8[:], in_=vcand[:])
                    eqm = mrg.tile([P, CAND * NQ], F32, tag="eqm", name="eqm")
                    nc.vector.tensor_tensor(
                        out=eqm[:].rearrange("p (a b) -> p a b", a=CAND),
                        in0=vcand[:].rearrange(
                            "p (a q) -> p a q", a=1).to_broadcast(
                            [P, CAND, NQ]),
                        in1=g8[:, 0:CAND].rearrange(
                            "p (a o) -> p a o", o=1).to_broadcast(
                            [P, CAND, NQ]),
                        op=OP.is_equal)
                    prod = mrg.tile([P, CAND * NQ], F32, tag="prod",
                                    name="prod")
                    nc.vector.tensor_tensor(
                        out=prod[:].rearrange("p (a b) -> p a b", a=CAND),
                        in0=eqm[:].rearrange("p (a b) -> p a b", a=CAND),
                        in1=icand[:].rearrange(
                            "p (a q) -> p a q", a=1).to_broadcast(
                            [P, CAND, NQ]),
                        op=OP.mult)
                    idx6f = mrg.tile([P, CAND], F32, tag="idx6f", name="idx6f")
                    nc.vector.tensor_reduce(
                        out=idx6f[:],
                        in_=prod[:].rearrange("p (a b) -> p a b", a=CAND),
                        axis=mybir.AxisListType.X, op=OP.add)
                    idx6 = mrg.tile([P, CAND], U32, tag="idx6", name="idx6")
                    nc.vector.tensor_copy(idx6[:], idx6f[:])

                    G = gat.tile([P, CAND * d], F32, tag="G", name="G")
                    for k in range(CAND):
                        nc.gpsimd.indirect_dma_start(
                            out=G[:, k * d:(k + 1) * d], out_offset=None,
                            in_=anchor[:],
                            in_offset=IndirectOffsetOnAxis(
                                ap=idx6[:, k:k + 1], axis=0))

                    # exact d2 for ranks 4..6 (slots 3..5); slots 0..2
                    # forced-selected (-1e30), 6..7 forced-out (+1e30)
                    d8 = mrg.tile([P, 8], F32, tag="d8", name="d8")
                    nc.vector.memset(d8[:], 1e30)
                    nc.vector.memset(d8[:, 0:3], -1e30)
                    accA = mrg.tile([P, d], F32, tag="accA", name="accA")
                    accB = mrg.tile([P, d], F32, tag="accB", name="accB")
                    sqdump = accB[:].bitcast(BF16)[:, 0:d]
                    for k in range(3, CAND):
                        diff = mrg.tile([P, d], F32, tag="diff", name="diff")
                        nc.vector.tensor_tensor(
                            out=diff[:], in0=sn[:],
                            in1=G[:, k * d:(k + 1) * d], op=OP.subtract)
                        nc.scalar.activation(sqdump, diff[:], AF.Square,
                                             accum_out=d8[:, k:k + 1])
                    m8 = mrg.tile([P, 8], F32, tag="m8", name="m8")
                    nc.vector.max(out=m8[:], in_=d8[:])
                    # 4th largest = 2nd smallest of the refined 3
                    w = mrg.tile([P, 8], F32, tag="w", name="w")
                    nc.vector.scalar_tensor_tensor(
                        out=w[:], in0=d8[:], scalar=m8[:, 3:4],
                        in1=ones8[:], op0=OP.is_le, op1=OP.mult)

                    mvt = mrg.tile([P, d], BF16, tag="mvt", name="mvt")
                    nc.vector.tensor_tensor(
                        out=accA[:].rearrange("p (a b) -> p a b", a=1),
                        in0=G[:, 0:d].rearrange("p (a b) -> p a b", a=1),
                        in1=w[:, 0:1].rearrange(
                            "p (a o) -> p a o", o=1).to_broadcast([P, 1, d]),
                        op=OP.mult)
                    accs = [None, accA, accB, accA, accB, accA]
                    for k in range(1, CAND):
                        dst = mvt if k == CAND - 1 else accs[k + 1]
                        nc.vector.scalar_tensor_tensor(
                            out=dst[:], in0=G[:, k * d:(k + 1) * d],
                            scalar=w[:, k:k + 1], in1=accs[k][:],
                            op0=OP.mult, op1=OP.add)
                    for j in range(DC):
                        nc.sync.dma_start_transpose(
                            out=neighT[j][:, t * P:(t + 1) * P],
                            in_=mvt[:, j * P:(j + 1) * P])

            anch_ctx.__exit__(None, None, None)

            # ================= PHASE B: dense chain =================

            def load_w(t_dram, rows, cols, tag):
                tiles = []
                for c in range(rows // P):
                    w_ = wp.tile([P, cols], BF16, tag=f"{tag}{c}",
                                 name=f"{tag}{c}")
                    nc.sync.dma_start(out=w_[:], in_=t_dram[c * P:(c + 1) * P, :])
                    tiles.append(w_)
                return tiles

            wdim_t = load_w(wdim, d, d, "wdim")
            wfus_t = load_w(wfus, 2 * d, d, "wfus")
            we1_t = load_w(we1, d, f, "we1")
            we2_t = load_w(we2, f, d, "we2")
            wd_t = load_w(wd, d, d, "wd")

            bias_t = {}
            for name, t_dram, cols in [
                    ("bdim", bdim, DC), ("bfus", bfus, DC), ("be1", be1, FC),
                    ("be2", be2, DC), ("bd", bd, DC), ("g1", g1, DC),
                    ("bt1", bt1, DC), ("g2", g2, DC), ("bt2", bt2, DC),
                    ("gd", gd, DC), ("btd", btd, DC)]:
                bt_ = wp.tile([P, cols], F32, tag=name, name=name)
                nc.sync.dma_start(out=bt_[:], in_=t_dram[:, :])
                bias_t[name] = bt_

            with (
                tc.tile_pool(name="act", bufs=1) as ap_,
                tc.tile_pool(name="mlp", bufs=1) as mp_,
                tc.tile_pool(name="bps", bufs=4, space="PSUM") as bps,
                tc.tile_pool(name="stat", bufs=1) as stp,
                tc.tile_pool(name="dram", bufs=1, space="DRAM") as _dp,
            ):
                amp_ctx = tc.tile_pool(name="amap", bufs=1)
                amp = amp_ctx.__enter__()
                amapT = [amp.tile([P, ns], BF16, tag=f"amap{c}", name=f"amap{c}")
                         for c in range(DC)]
                for nb in range(NB):
                    n_sl = slice(nb * nbf, (nb + 1) * nbf)
                    for fc in range(DC):
                        ps = bps.tile([P, nbf], F32, tag="psB", name="psB")
                        for c in range(DC):
                            nc.tensor.matmul(
                                ps[:], wdim_t[c][:, fc * P:(fc + 1) * P],
                                neighT[c][:, n_sl],
                                start=(c == 0), stop=(c == DC - 1))
                        nc.scalar.activation(amapT[fc][:, n_sl], ps[:],
                                             AF.Identity,
                                             bias=bias_t["bdim"][:, fc:fc + 1])

                combraw = [ap_.tile([P, ns], BF16, tag=f"craw{c}", name=f"craw{c}")
                           for c in range(DC)]
                for nb in range(NB):
                    n_sl = slice(nb * nbf, (nb + 1) * nbf)
                    for fc in range(DC):
                        ps = bps.tile([P, nbf], F32, tag="psB", name="psB")
                        for c in range(2 * DC):
                            rhs = sTh[c][:, n_sl] if c < DC else \
                                amapT[c - DC][:, n_sl]
                            nc.tensor.matmul(
                                ps[:], wfus_t[c][:, fc * P:(fc + 1) * P], rhs,
                                start=(c == 0), stop=(c == 2 * DC - 1))
                        nc.scalar.activation(combraw[fc][:, n_sl], ps[:],
                                             AF.Identity,
                                             bias=bias_t["bfus"][:, fc:fc + 1])

                amp_ctx.__exit__(None, None, None)

                def bn_stats(tiles, idx):
                    st = stp.tile([P, 2 * DC], F32, tag=f"st{idx}", name=f"st{idx}")
                    scr = stp.tile([P, ns], BF16, tag="sq_scratch",
                                   name="sq_scratch")
                    for c in range(DC):
                        nc.vector.tensor_reduce(out=st[:, c:c + 1],
                                                in_=tiles[c][:],
                                                axis=mybir.AxisListType.X,
                                                op=OP.add)
                        nc.scalar.activation(scr[:], tiles[c][:], AF.Square,
                                             accum_out=st[:, DC + c:DC + c + 1])
                    nc.sync.dma_start(out=cc_in[idx][:], in_=st[:])
                    nc.gpsimd.collective_compute(
                        "AllReduce", OP.add, replica_groups=groups,
                        ins=[cc_in[idx].ap()], outs=[cc_out[idx].ap()])
                    gst = stp.tile([P, 2 * DC], F32, tag=f"gst{idx}", name=f"gst{idx}")
                    nc.sync.dma_start(out=gst[:], in_=cc_out[idx][:])
                    # mu, var=E[x^2]-mu^2, s=g/sqrt(var+eps), t=beta-mu*s
                    mu = stp.tile([P, DC], F32, tag=f"mu{idx}", name=f"mu{idx}")
                    nc.vector.tensor_scalar(out=mu[:], in0=gst[:, :DC],
                                            scalar1=1.0 / NTOT, scalar2=None,
                                            op0=OP.mult)
                    musq = stp.tile([P, DC], F32, tag=f"musq{idx}", name=f"musq{idx}")
                    nc.vector.tensor_tensor(out=musq[:], in0=mu[:], in1=mu[:],
                                            op=OP.mult)
                    var = stp.tile([P, DC], F32, tag=f"var{idx}", name=f"var{idx}")
                    nc.vector.scalar_tensor_tensor(
                        out=var[:], in0=gst[:, DC:], scalar=1.0 / NTOT,
                        in1=musq[:], op0=OP.mult, op1=OP.subtract)
                    sd = stp.tile([P, DC], F32, tag=f"sd{idx}", name=f"sd{idx}")
                    nc.vector.tensor_scalar(out=sd[:], in0=var[:], scalar1=EPS,
                                            scalar2=None, op0=OP.add)
                    nc.scalar.sqrt(sd[:], sd[:])
                    rs = stp.tile([P, DC], F32, tag=f"rs{idx}", name=f"rs{idx}")
                    nc.vector.reciprocal(rs[:], sd[:])
                    return mu, rs

                def bn_affine(mu, rs, gname, bname, idx):
                    s = stp.tile([P, DC], F32, tag=f"s{idx}", name=f"s{idx}")
                    nc.vector.tensor_tensor(out=s[:], in0=rs[:],
                                            in1=bias_t[gname][:], op=OP.mult)
                    tmp = stp.tile([P, DC], F32, tag=f"tmp{idx}", name=f"tmp{idx}")
                    nc.vector.tensor_tensor(out=tmp[:], in0=mu[:], in1=s[:],
                                            op=OP.mult)
                    tb = stp.tile([P, DC], F32, tag=f"tb{idx}", name=f"tb{idx}")
                    nc.vector.tensor_tensor(out=tb[:], in0=bias_t[bname][:],
                                            in1=tmp[:], op=OP.subtract)
                    return s, tb

                mu1, rs1 = bn_stats(combraw, 0)
                s1, t1 = bn_affine(mu1, rs1, "g1", "bt1", 0)
                combT = [ap_.tile([P, ns], BF16, tag=f"combT{c}", name=f"combT{c}")
                         for c in range(DC)]
                for c in range(DC):
                    nc.scalar.activation(combT[c][:], combraw[c][:],
                                         AF.Identity, bias=t1[:, c:c + 1],
                                         scale=s1[:, c:c + 1])

                r2T = [ap_.tile([P, ns], BF16, tag=f"r2T{c}", name=f"r2T{c}")
                       for c in range(DC)]
                for nb in range(NB):
                    n_sl = slice(nb * nbf, (nb + 1) * nbf)
                    tT = [mp_.tile([P, nbf], BF16, tag=f"tT{fe}", name=f"tT{fe}")
                          for fe in range(FC)]
                    for fe in range(FC):
                        ps = bps.tile([P, nbf], F32, tag="psB", name="psB")
                        for c in range(DC):
                            nc.tensor.matmul(
                                ps[:], we1_t[c][:, fe * P:(fe + 1) * P],
                                combT[c][:, n_sl],
                                start=(c == 0), stop=(c == DC - 1))
                        nc.scalar.activation(tT[fe][:], ps[:], AF.Tanh,
                                             bias=bias_t["be1"][:, fe:fe + 1])
                    for fc in range(DC):
                        ps = bps.tile([P, nbf], F32, tag="psB", name="psB")
                        for fe in range(FC):
                            nc.tensor.matmul(
                                ps[:], we2_t[fe][:, fc * P:(fc + 1) * P],
                                tT[fe][:],
                                start=(fe == 0), stop=(fe == FC - 1))
                        # r2 = (psum + b_e2) + comb  (residual, bias fused)
                        nc.vector.scalar_tensor_tensor(
                            out=r2T[fc][:, n_sl], in0=ps[:],
                            scalar=bias_t["be2"][:, fc:fc + 1],
                            in1=combT[fc][:, n_sl], op0=OP.add, op1=OP.add)

                mu2, rs2 = bn_stats(r2T, 1)
                s2, t2 = bn_affine(mu2, rs2, "g2", "bt2", 1)
                c2T = combraw  # reuse buffers
                for c in range(DC):
                    nc.scalar.activation(c2T[c][:], r2T[c][:], AF.Identity,
                                         bias=t2[:, c:c + 1],
                                         scale=s2[:, c:c + 1])

                yT = [ap_.tile([P, ns], BF16, tag=f"yT{c}", name=f"yT{c}")
                      for c in range(DC)]
                for nb in range(NB):
                    n_sl = slice(nb * nbf, (nb + 1) * nbf)
                    for fc in range(DC):
                        ps = bps.tile([P, nbf], F32, tag="psB", name="psB")
                        for c in range(DC):
                            nc.tensor.matmul(
                                ps[:], wd_t[c][:, fc * P:(fc + 1) * P],
                                c2T[c][:, n_sl],
                                start=(c == 0), stop=(c == DC - 1))
                        nc.scalar.activation(yT[fc][:, n_sl], ps[:],
                                             AF.Identity,
                                             bias=bias_t["bd"][:, fc:fc + 1])

                mu3, rs3 = bn_stats(yT, 2)
                s3, t3 = bn_affine(mu3, rs3, "gd", "btd", 2)

                # fused BN3+tanh, transpose back to [ns, d], store
                with (
                    tc.tile_pool(name="ops", bufs=2, space="PSUM") as opsp,
                    tc.tile_pool(name="onat", bufs=3) as onp,
                ):
                    for t in range(T):
                        otmp = onp.tile([P, d], F32, tag="otmp", name="otmp")
                        for j in range(DC):
                            nc.scalar.activation(
                                otmp[:, j * P:(j + 1) * P],
                                yT[j][:, t * P:(t + 1) * P], AF.Tanh,
                                bias=t3[:, j:j + 1], scale=s3[:, j:j + 1])
                        tps = opsp.tile([P, d], F32, name="otps")
                        for j in range(DC):
                            nc.tensor.transpose(
                                out=tps[:, j * P:(j + 1) * P],
                                in_=otmp[:, j * P:(j + 1) * P],
                                identity=ident[:])
                        onat = onp.tile([P, d], F32, tag="onat", name="onat")
                        nc.scalar.copy(onat[:], tps[:])
                        nc.sync.dma_start(out=out[t * P:(t + 1) * P, :],
                                          in_=onat[:])

    nc.finalize()
    return nc


def _chunk_vec(v, cols):
    # [cols*128] feature vector -> [128, cols] feature-major chunk layout
    return np.ascontiguousarray(v.reshape(cols, P).T)


def prepare_inputs(src, anchor_2, W_dim, b_dim, W_fus, b_fus, W_e1, b_e1,
                   W_e2, b_e2, g1, bt1, g2, bt2, W_d, b_d, g_d, bt_d,
                   n_cores=N_CORES, ns=N_FULL // N_CORES):
    """Host-side prep: shard + transpose + bf16 casts + layout transforms."""
    d = src.shape[1]
    f = W_e1.shape[1]
    m = anchor_2.shape[0]
    DC, FC = d // P, f // P
    am2 = (anchor_2.astype(np.float64) ** 2).sum(1)
    c = 256.0 - 0.5 * am2
    ch = c.astype(np.float32).astype(ml_dtypes.bfloat16)
    cl = (c - ch.astype(np.float64)).astype(np.float32).astype(
        ml_dtypes.bfloat16)
    caug = np.zeros((4, m), dtype=ml_dtypes.bfloat16)
    caug[0] = ch
    caug[1] = cl
    shared = dict(
        anchT_h=anchor_2.T.astype(ml_dtypes.bfloat16),
        caug=caug,
        anchor=np.ascontiguousarray(anchor_2),
        wdim=(W_dim / K).astype(ml_dtypes.bfloat16),
        wfus=W_fus.astype(ml_dtypes.bfloat16),
        we1=W_e1.astype(ml_dtypes.bfloat16),
        we2=W_e2.astype(ml_dtypes.bfloat16),
        wd=W_d.astype(ml_dtypes.bfloat16),
        bdim=_chunk_vec(b_dim, DC), bfus=_chunk_vec(b_fus, DC),
        be1=_chunk_vec(b_e1, FC), be2=_chunk_vec(b_e2, DC),
        bd=_chunk_vec(b_d, DC),
        g1=_chunk_vec(g1, DC), bt1=_chunk_vec(bt1, DC),
        g2=_chunk_vec(g2, DC), bt2=_chunk_vec(bt2, DC),
        gd=_chunk_vec(g_d, DC), btd=_chunk_vec(bt_d, DC),
    )
    in_maps = []
    for cix in range(n_cores):
        shard = np.ascontiguousarray(src[cix * ns:(cix + 1) * ns])
        in_maps.append(dict(
            shared,
            srcT_h=shard.T.astype(ml_dtypes.bfloat16),
            src_nat=shard.astype(np.float32)))
    return in_maps


_NC_CACHE = {}


def kernel(**inputs):
    key = "full"
    if key not in _NC_CACHE:
        _NC_CACHE[key] = build_kernel()
    nc = _NC_CACHE[key]
    in_maps = prepare_inputs(**{k: np.asarray(v) for k, v in inputs.items()})
    res = run_bass_kernel_spmd(nc, in_maps, core_ids=list(range(N_CORES)))
    return np.concatenate([r["out"] for r in res.results], axis=0)


# revision 11
# speedup vs baseline: 1.0813x; 1.0813x over previous
"""Trainium2 Bass kernel for nn_Encoder_61830349193463 (retrieval_knn).

v3 strategy (data-parallel over src rows, 8 NeuronCores):
  - Each core gets a 2048-row shard of src; anchors + weights replicated.
  - kNN sims via a SINGLE bf16 PE matmul pass:
      sim = sh.ah + (256 - 0.5*||a||^2)  [bias folded in as split-bf16
      rows of a K=4 ones matmul]
  - All 4 anchor chunks ([128, 8192] bf16 each) stay RESIDENT in SBUF, so
    the loop runs tile-major: for each 128-row tile, 4 quarter matmuls into
    PSUM with max8 + find_index8 scans DIRECTLY on PSUM, then the
    candidate post-processing (merge -> gather -> exact refine -> top-5
    mask -> mean) runs inline and pipelines against the next tile's
    matmuls/scans (keeps the PE warm; v2 serialized this after all sims).
  - Top-6 anchors gathered fp32; ranks 4..6 re-scored EXACTLY as
    d2 = sum((s-g)^2): DVE subtract + ScalarE Square-accumulate; best 2 of
    the refined 3 + unconditional top-3 = exact top-5 (mask via is_le
    against the 4th-largest of a padded 8-slot score row).
  - mean(top5) = sum_k w_k * G_k via 6 chained DVE stt ops (w in {0,1},
    the /K folded into W_dim host-side); transpose to feature-major
    deferred to phase B (PSUM fully owned by the sims in phase A).
  - Dense chain: feature-major, BN stats AllReduced (3 tiny collectives),
    all matmuls bf16.
"""

import numpy as np

import concourse.bacc as bacc
import concourse.bass as bass
import concourse.mybir as mybir
import concourse.tile as tile
from concourse.bass import IndirectOffsetOnAxis
from concourse.bass_utils import run_bass_kernel_spmd
from concourse.masks import make_identity
import ml_dtypes

F32 = mybir.dt.float32
BF16 = mybir.dt.bfloat16
U32 = mybir.dt.uint32
AF = mybir.ActivationFunctionType
OP = mybir.AluOpType
P = 128

# problem sizes (hardcoded per contract)
N_FULL, M, D, F = 16384, 8192, 512, 2048
N_CORES = 8
K = 5
EPS = 1e-5
CAND = 6            # gathered candidates; ranks 4..6 exactly re-scored


def build_kernel(ns=N_FULL // N_CORES, m=M, d=D, f=F, n_cores=N_CORES,
                 mc_free=512, q_div=4):
    """Build the SPMD Bass module. ns/m/d/f sizes are per-core."""
    DC = d // P          # contraction chunks of the d dim (4)
    FC = f // P          # chunks of the hidden dim (16)
    T = ns // P          # n-tiles per core (16)
    nbf = min(mc_free, ns)
    NB = ns // nbf       # n blocks of 512 for phase-B matmuls (4)
    MQ = m // q_div      # m-quarter size (2048)
    QC = MQ // mc_free   # 512-chunks per quarter (4)
    NQ = 8 * q_div       # merged candidate pool width (32)
    NTOT = float(ns * n_cores)

    nc = bacc.Bacc("TRN2", target_bir_lowering=False, debug=False,
                   num_devices=n_cores)

    def param(name, shape, dt=F32):
        return nc.declare_dram_parameter(name, list(shape), dt, isOutput=False)

    srcT_h = param("srcT_h", [d, ns], BF16)
    src_nat = param("src_nat", [ns, d], F32)
    anchT_h = param("anchT_h", [d, m], BF16)
    caug = param("caug", [4, m], BF16)             # rows: c_h, c_l, 0, 0
    anchor = param("anchor", [m, d], F32)          # natural, for the gather
    wdim = param("wdim", [d, d], BF16)             # pre-scaled by 1/K
    wfus = param("wfus", [2 * d, d], BF16)
    we1 = param("we1", [d, f], BF16)
    we2 = param("we2", [f, d], BF16)
    wd = param("wd", [d, d], BF16)
    bdim = param("bdim", [P, DC])
    bfus = param("bfus", [P, DC])
    be1 = param("be1", [P, FC])
    be2 = param("be2", [P, DC])
    bd = param("bd", [P, DC])
    g1 = param("g1", [P, DC]); bt1 = param("bt1", [P, DC])
    g2 = param("g2", [P, DC]); bt2 = param("bt2", [P, DC])
    gd = param("gd", [P, DC]); btd = param("btd", [P, DC])
    out = nc.declare_dram_parameter("out", [ns, d], F32, isOutput=True)

    # internal DRAM for the three BN-stat AllReduces
    cc_in = [nc.dram_tensor(f"cc{i}_in", [P, 2 * DC], F32) for i in range(3)]
    cc_space = "Shared" if n_cores > 4 else "Local"
    cc_out = [nc.dram_tensor(f"cc{i}_out", [P, 2 * DC], F32,
                             addr_space=cc_space) for i in range(3)]
    groups = [list(range(n_cores))]

    with tile.TileContext(nc) as tc:
        with (
            tc.tile_pool(name="persist", bufs=1) as pp,
            tc.tile_pool(name="wpool", bufs=1) as wp,
        ):
            ident = pp.tile([P, P], F32, name="ident")
            make_identity(nc, ident[:])
            ones4 = pp.tile([4, P], BF16, name="ones4")
            nc.vector.memset(ones4[:], 1.0)
            ones8 = pp.tile([P, 8], F32, name="ones8")
            nc.vector.memset(ones8[:], 1.0)

            # resident bf16 source (hi split only)
            sTh = []
            for c in range(DC):
                th = pp.tile([P, ns], BF16, tag=f"sTh{c}", name=f"sTh{c}")
                nc.sync.dma_start(out=th[:], in_=srcT_h[c * P:(c + 1) * P, :])
                sTh.append(th)

            # neighbour means go to a DRAM scratch (read back in phase B)
            meanv_dram = nc.dram_tensor("meanv_scratch", [ns, d], BF16)
            identb = pp.tile([P, P], BF16, name="identb")
            nc.vector.tensor_copy(identb[:], ident[:])

            # ================= PHASE A: kNN =================
            anch_ctx = tc.tile_pool(name="anch", bufs=1)
            anp = anch_ctx.__enter__()
            aT = []
            for c in range(DC):
                at = anp.tile([P, m], BF16, tag=f"aT{c}", name=f"aT{c}")
                nc.sync.dma_start(out=at[:], in_=anchT_h[c * P:(c + 1) * P, :])
                aT.append(at)
            caugt = anp.tile([4, m], BF16, tag="caug", name="caug")
            nc.sync.dma_start(out=caugt[:], in_=caug[:, :])

            with (
                tc.tile_pool(name="dps", bufs=2, space="PSUM") as dps,
                tc.tile_pool(name="tops", bufs=3) as tops,
                tc.tile_pool(name="mrg", bufs=2) as mrg,
                tc.tile_pool(name="gat", bufs=2) as gat,
                tc.tile_pool(name="snp", bufs=2) as snp,
            ):
                for t in range(T):
                    n_sl = slice(t * P, (t + 1) * P)
                    sn = snp.tile([P, d], F32, tag="sn", name="sn")
                    nc.sync.dma_start(out=sn[:],
                                      in_=src_nat[t * P:(t + 1) * P, :])
                    vcand = mrg.tile([P, NQ], F32, tag="vcand", name="vcand")
                    icand = mrg.tile([P, NQ], F32, tag="icand", name="icand")
                    for q in range(q_div):
                        ps = dps.tile([P, MQ], F32, name="dps")
                        for c in range(DC):
                            for mc in range(QC):
                                m_sl = slice(q * MQ + mc * mc_free,
                                             q * MQ + (mc + 1) * mc_free)
                                p_sl = slice(mc * mc_free, (mc + 1) * mc_free)
                                nc.tensor.matmul(ps[:, p_sl], sTh[c][:, n_sl],
                                                 aT[c][:, m_sl],
                                                 start=(c == 0), stop=False)
                        for mc in range(QC):
                            m_sl = slice(q * MQ + mc * mc_free,
                                         q * MQ + (mc + 1) * mc_free)
                            p_sl = slice(mc * mc_free, (mc + 1) * mc_free)
                            nc.tensor.matmul(ps[:, p_sl], ones4[:],
                                             caugt[:, m_sl],
                                             start=False, stop=True)
                        v8 = vcand[:, q * 8:(q + 1) * 8]
                        nc.vector.max(out=v8, in_=ps[:])
                        i8 = tops.tile([P, 8], U32, tag="i8", name="i8")
                        nc.vector.max_index(out=i8[:], in_max=v8,
                                            in_values=ps[:])
                        nc.vector.tensor_scalar(
                            out=icand[:, q * 8:(q + 1) * 8], in0=i8[:],
                            scalar1=float(q * MQ), scalar2=None, op0=OP.add)

                    # ---- merge quarters to global top-6 ----
                    g8 = mrg.tile([P, 8], F32, tag="g8", name="g8")
                    nc.vector.max(out=g8[:], in_=vcand[:])
                    eqm = mrg.tile([P, CAND * NQ], F32, tag="eqm", name="eqm")
                    nc.vector.tensor_tensor(
                        out=eqm[:].rearrange("p (a b) -> p a b", a=CAND),
                        in0=vcand[:].rearrange(
                            "p (a q) -> p a q", a=1).to_broadcast(
                            [P, CAND, NQ]),
                        in1=g8[:, 0:CAND].rearrange(
                            "p (a o) -> p a o", o=1).to_broadcast(
                            [P, CAND, NQ]),
                        op=OP.is_equal)
                    prod = mrg.tile([P, CAND * NQ], F32, tag="prod",
                                    name="prod")
                    nc.vector.tensor_tensor(
                        out=prod[:].rearrange("p (a b) -> p a b", a=CAND),
                        in0=eqm[:].rearrange("p (a b) -> p a b", a=CAND),
                        in1=icand[:].rearrange(
                            "p (a q) -> p a q", a=1).to_broadcast(
                            [P, CAND, NQ]),
                        op=OP.mult)
                    idx6f = mrg.tile([P, CAND], F32, tag="idx6f", name="idx6f")
                    nc.vector.tensor_reduce(
                        out=idx6f[:],
                        in_=prod[:].rearrange("p (a b) -> p a b", a=CAND),
                        axis=mybir.AxisListType.X, op=OP.add)
                    idx6 = mrg.tile([P, CAND], U32, tag="idx6", name="idx6")
                    nc.vector.tensor_copy(idx6[:], idx6f[:])

                    G = gat.tile([P, CAND * d], F32, tag="G", name="G")
                    for k in range(CAND):
                        nc.gpsimd.indirect_dma_start(
                            out=G[:, k * d:(k + 1) * d], out_offset=None,
                            in_=anchor[:],
                            in_offset=IndirectOffsetOnAxis(
                                ap=idx6[:, k:k + 1], axis=0))

                    # exact d2 for ranks 4..6 (slots 3..5); slots 0..2
                    # forced-selected (-1e30), 6..7 forced-out (+1e30)
                    d8 = mrg.tile([P, 8], F32, tag="d8", name="d8")
                    nc.vector.memset(d8[:], 1e30)
                    nc.vector.memset(d8[:, 0:3], -1e30)
                    accA = mrg.tile([P, d], F32, tag="accA", name="accA")
                    accB = mrg.tile([P, d], F32, tag="accB", name="accB")
                    sqdump = accB[:].bitcast(BF16)[:, 0:d]
                    for k in range(3, CAND):
                        diff = mrg.tile([P, d], F32, tag="diff", name="diff")
                        nc.vector.tensor_tensor(
                            out=diff[:], in0=sn[:],
                            in1=G[:, k * d:(k + 1) * d], op=OP.subtract)
                        nc.scalar.activation(sqdump, diff[:], AF.Square,
                                             accum_out=d8[:, k:k + 1])
                    m8 = mrg.tile([P, 8], F32, tag="m8", name="m8")
                    nc.vector.max(out=m8[:], in_=d8[:])
                    # 4th largest = 2nd smallest of the refined 3
                    w = mrg.tile([P, 8], F32, tag="w", name="w")
                    nc.vector.scalar_tensor_tensor(
                        out=w[:], in0=d8[:], scalar=m8[:, 3:4],
                        in1=ones8[:], op0=OP.is_le, op1=OP.mult)

                    mvt = mrg.tile([P, d], BF16, tag="mvt", name="mvt")
                    nc.vector.tensor_tensor(
                        out=accA[:].rearrange("p (a b) -> p a b", a=1),
                        in0=G[:, 0:d].rearrange("p (a b) -> p a b", a=1),
                        in1=w[:, 0:1].rearrange(
                            "p (a o) -> p a o", o=1).to_broadcast([P, 1, d]),
                        op=OP.mult)
                    accs = [None, accA, accB, accA, accB, accA]
                    for k in range(1, CAND):
                        dst = mvt if k == CAND - 1 else accs[k + 1]
                        nc.vector.scalar_tensor_tensor(
                            out=dst[:], in0=G[:, k * d:(k + 1) * d],
                            scalar=w[:, k:k + 1], in1=accs[k][:],
                            op0=OP.mult, op1=OP.add)
                    nc.sync.dma_start(out=meanv_dram[t * P:(t + 1) * P, :],
                                      in_=mvt[:])

            anch_ctx.__exit__(None, None, None)

            # ================= PHASE B: dense chain =================
            neighT = [pp.tile([P, ns], BF16, tag=f"nT{c}", name=f"nT{c}")
                      for c in range(DC)]
            with (
                tc.tile_pool(name="tps", bufs=2, space="PSUM") as tpsp,
                tc.tile_pool(name="mvload", bufs=2) as mvl,
            ):
                for t in range(T):
                    mvt = mvl.tile([P, d], BF16, tag="mv", name="mv")
                    nc.sync.dma_start(out=mvt[:],
                                      in_=meanv_dram[t * P:(t + 1) * P, :])
                    tps = tpsp.tile([P, d], BF16, name="tps")
                    for j in range(DC):
                        nc.tensor.transpose(
                            out=tps[:, j * P:(j + 1) * P],
                            in_=mvt[:, j * P:(j + 1) * P],
                            identity=identb[:])
                    for j in range(DC):
                        nc.scalar.copy(neighT[j][:, t * P:(t + 1) * P],
                                       tps[:, j * P:(j + 1) * P])

            def load_w(t_dram, rows, cols, tag):
                tiles = []
                for c in range(rows // P):
                    w_ = wp.tile([P, cols], BF16, tag=f"{tag}{c}",
                                 name=f"{tag}{c}")
                    nc.sync.dma_start(out=w_[:], in_=t_dram[c * P:(c + 1) * P, :])
                    tiles.append(w_)
                return tiles

            wdim_t = load_w(wdim, d, d, "wdim")
            wfus_t = load_w(wfus, 2 * d, d, "wfus")
            we1_t = load_w(we1, d, f, "we1")
            we2_t = load_w(we2, f, d, "we2")
            wd_t = load_w(wd, d, d, "wd")

            bias_t = {}
            for name, t_dram, cols in [
                    ("bdim", bdim, DC), ("bfus", bfus, DC), ("be1", be1, FC),
                    ("be2", be2, DC), ("bd", bd, DC), ("g1", g1, DC),
                    ("bt1", bt1, DC), ("g2", g2, DC), ("bt2", bt2, DC),
                    ("gd", gd, DC), ("btd", btd, DC)]:
                bt_ = wp.tile([P, cols], F32, tag=name, name=name)
                nc.sync.dma_start(out=bt_[:], in_=t_dram[:, :])
                bias_t[name] = bt_

            with (
                tc.tile_pool(name="act", bufs=1) as ap_,
                tc.tile_pool(name="mlp", bufs=1) as mp_,
                tc.tile_pool(name="bps", bufs=4, space="PSUM") as bps,
                tc.tile_pool(name="stat", bufs=1) as stp,
                tc.tile_pool(name="dram", bufs=1, space="DRAM") as _dp,
            ):
                amp_ctx = tc.tile_pool(name="amap", bufs=1)
                amp = amp_ctx.__enter__()
                amapT = [amp.tile([P, ns], BF16, tag=f"amap{c}", name=f"amap{c}")
                         for c in range(DC)]
                for nb in range(NB):
                    n_sl = slice(nb * nbf, (nb + 1) * nbf)
                    for fc in range(DC):
                        ps = bps.tile([P, nbf], F32, tag="psB", name="psB")
                        for c in range(DC):
                            nc.tensor.matmul(
                                ps[:], wdim_t[c][:, fc * P:(fc + 1) * P],
                                neighT[c][:, n_sl],
                                start=(c == 0), stop=(c == DC - 1))
                        nc.scalar.activation(amapT[fc][:, n_sl], ps[:],
                                             AF.Identity,
                                             bias=bias_t["bdim"][:, fc:fc + 1])

                combraw = [ap_.tile([P, ns], BF16, tag=f"craw{c}", name=f"craw{c}")
                           for c in range(DC)]
                for fc in range(DC):
                    for nb in range(NB):
                        n_sl = slice(nb * nbf, (nb + 1) * nbf)
                        ps = bps.tile([P, nbf], F32, tag="psB", name="psB")
                        for c in range(2 * DC):
                            rhs = sTh[c][:, n_sl] if c < DC else \
                                amapT[c - DC][:, n_sl]
                            nc.tensor.matmul(
                                ps[:], wfus_t[c][:, fc * P:(fc + 1) * P], rhs,
                                start=(c == 0), stop=(c == 2 * DC - 1))
                        nc.scalar.activation(combraw[fc][:, n_sl], ps[:],
                                             AF.Identity,
                                             bias=bias_t["bfus"][:, fc:fc + 1])

                amp_ctx.__exit__(None, None, None)

                def bn_stats(tiles, idx):
                    st = stp.tile([P, 2 * DC], F32, tag=f"st{idx}", name=f"st{idx}")
                    scr = stp.tile([P, ns], BF16, tag="sq_scratch",
                                   name="sq_scratch")
                    for c in range(DC):
                        nc.vector.tensor_reduce(out=st[:, c:c + 1],
                                                in_=tiles[c][:],
                                                axis=mybir.AxisListType.X,
                                                op=OP.add)
                        nc.scalar.activation(scr[:], tiles[c][:], AF.Square,
                                             accum_out=st[:, DC + c:DC + c + 1])
                    nc.sync.dma_start(out=cc_in[idx][:], in_=st[:])
                    nc.gpsimd.collective_compute(
                        "AllReduce", OP.add, replica_groups=groups,
                        ins=[cc_in[idx].ap()], outs=[cc_out[idx].ap()])
                    gst = stp.tile([P, 2 * DC], F32, tag=f"gst{idx}", name=f"gst{idx}")
                    nc.sync.dma_start(out=gst[:], in_=cc_out[idx][:])
                    # mu, var=E[x^2]-mu^2, s=g/sqrt(var+eps), t=beta-mu*s
                    mu = stp.tile([P, DC], F32, tag=f"mu{idx}", name=f"mu{idx}")
                    nc.vector.tensor_scalar(out=mu[:], in0=gst[:, :DC],
                                            scalar1=1.0 / NTOT, scalar2=None,
                                            op0=OP.mult)
                    musq = stp.tile([P, DC], F32, tag=f"musq{idx}", name=f"musq{idx}")
                    nc.vector.tensor_tensor(out=musq[:], in0=mu[:], in1=mu[:],
                                            op=OP.mult)
                    var = stp.tile([P, DC], F32, tag=f"var{idx}", name=f"var{idx}")
                    nc.vector.scalar_tensor_tensor(
                        out=var[:], in0=gst[:, DC:], scalar=1.0 / NTOT,
                        in1=musq[:], op0=OP.mult, op1=OP.subtract)
                    sd = stp.tile([P, DC], F32, tag=f"sd{idx}", name=f"sd{idx}")
                    nc.vector.tensor_scalar(out=sd[:], in0=var[:], scalar1=EPS,
                                            scalar2=None, op0=OP.add)
                    nc.scalar.sqrt(sd[:], sd[:])
                    rs = stp.tile([P, DC], F32, tag=f"rs{idx}", name=f"rs{idx}")
                    nc.vector.reciprocal(rs[:], sd[:])
                    return mu, rs

                def bn_affine(mu, rs, gname, bname, idx):
                    s = stp.tile([P, DC], F32, tag=f"s{idx}", name=f"s{idx}")
                    nc.vector.tensor_tensor(out=s[:], in0=rs[:],
                                            in1=bias_t[gname][:], op=OP.mult)
                    tmp = stp.tile([P, DC], F32, tag=f"tmp{idx}", name=f"tmp{idx}")
                    nc.vector.tensor_tensor(out=tmp[:], in0=mu[:], in1=s[:],
                                            op=OP.mult)
                    tb = stp.tile([P, DC], F32, tag=f"tb{idx}", name=f"tb{idx}")
                    nc.vector.tensor_tensor(out=tb[:], in0=bias_t[bname][:],
                                            in1=tmp[:], op=OP.subtract)
                    return s, tb

                mu1, rs1 = bn_stats(combraw, 0)
                s1, t1 = bn_affine(mu1, rs1, "g1", "bt1", 0)
                combT = [ap_.tile([P, ns], BF16, tag=f"combT{c}", name=f"combT{c}")
                         for c in range(DC)]
                for c in range(DC):
                    nc.scalar.activation(combT[c][:], combraw[c][:],
                                         AF.Identity, bias=t1[:, c:c + 1],
                                         scale=s1[:, c:c + 1])

                r2T = [ap_.tile([P, ns], BF16, tag=f"r2T{c}", name=f"r2T{c}")
                       for c in range(DC)]
                for nb in range(NB):
                    n_sl = slice(nb * nbf, (nb + 1) * nbf)
                    tT = [mp_.tile([P, nbf], BF16, tag=f"tT{fe}", name=f"tT{fe}")
                          for fe in range(FC)]
                    for fe in range(FC):
                        ps = bps.tile([P, nbf], F32, tag="psB", name="psB")
                        for c in range(DC):
                            nc.tensor.matmul(
                                ps[:], we1_t[c][:, fe * P:(fe + 1) * P],
                                combT[c][:, n_sl],
                                start=(c == 0), stop=(c == DC - 1))
                        nc.scalar.activation(tT[fe][:], ps[:], AF.Tanh,
                                             bias=bias_t["be1"][:, fe:fe + 1])
                    for fc in range(DC):
                        ps = bps.tile([P, nbf], F32, tag="psB", name="psB")
                        for fe in range(FC):
                            nc.tensor.matmul(
                                ps[:], we2_t[fe][:, fc * P:(fc + 1) * P],
                                tT[fe][:],
                                start=(fe == 0), stop=(fe == FC - 1))
                        # r2 = (psum + b_e2) + comb  (residual, bias fused)
                        nc.vector.scalar_tensor_tensor(
                            out=r2T[fc][:, n_sl], in0=ps[:],
                            scalar=bias_t["be2"][:, fc:fc + 1],
                            in1=combT[fc][:, n_sl], op0=OP.add, op1=OP.add)

                mu2, rs2 = bn_stats(r2T, 1)
                s2, t2 = bn_affine(mu2, rs2, "g2", "bt2", 1)
                c2T = combraw  # reuse buffers
                for c in range(DC):
                    nc.scalar.activation(c2T[c][:], r2T[c][:], AF.Identity,
                                         bias=t2[:, c:c + 1],
                                         scale=s2[:, c:c + 1])

                yT = [ap_.tile([P, ns], BF16, tag=f"yT{c}", name=f"yT{c}")
                      for c in range(DC)]
                for fc in range(DC):
                    for nb in range(NB):
                        n_sl = slice(nb * nbf, (nb + 1) * nbf)
                        ps = bps.tile([P, nbf], F32, tag="psB", name="psB")
                        for c in range(DC):
                            nc.tensor.matmul(
                                ps[:], wd_t[c][:, fc * P:(fc + 1) * P],
                                c2T[c][:, n_sl],
                                start=(c == 0), stop=(c == DC - 1))
                        nc.scalar.activation(yT[fc][:, n_sl], ps[:],
                                             AF.Identity,
                                             bias=bias_t["bd"][:, fc:fc + 1])

                mu3, rs3 = bn_stats(yT, 2)
                s3, t3 = bn_affine(mu3, rs3, "gd", "btd", 2)

                # fused BN3+tanh, transpose back to [ns, d], store
                with (
                    tc.tile_pool(name="ops", bufs=2, space="PSUM") as opsp,
                    tc.tile_pool(name="onat", bufs=3) as onp,
                ):
                    for t in range(T):
                        otmp = onp.tile([P, d], F32, tag="otmp", name="otmp")
                        for j in range(DC):
                            nc.scalar.activation(
                                otmp[:, j * P:(j + 1) * P],
                                yT[j][:, t * P:(t + 1) * P], AF.Tanh,
                                bias=t3[:, j:j + 1], scale=s3[:, j:j + 1])
                        tps = opsp.tile([P, d], F32, name="otps")
                        for j in range(DC):
                            nc.tensor.transpose(
                                out=tps[:, j * P:(j + 1) * P],
                                in_=otmp[:, j * P:(j + 1) * P],
                                identity=ident[:])
                        onat = onp.tile([P, d], F32, tag="onat", name="onat")
                        nc.scalar.copy(onat[:], tps[:])
                        nc.sync.dma_start(out=out[t * P:(t + 1) * P, :],
                                          in_=onat[:])

    nc.finalize()
    return nc


def _chunk_vec(v, cols):
    # [cols*128] feature vector -> [128, cols] feature-major chunk layout
    return np.ascontiguousarray(v.reshape(cols, P).T)


def prepare_inputs(src, anchor_2, W_dim, b_dim, W_fus, b_fus, W_e1, b_e1,
                   W_e2, b_e2, g1, bt1, g2, bt2, W_d, b_d, g_d, bt_d,
                   n_cores=N_CORES, ns=N_FULL // N_CORES):
    """Host-side prep: shard + transpose + bf16 casts + layout transforms."""
    d = src.shape[1]
    f = W_e1.shape[1]
    m = anchor_2.shape[0]
    DC, FC = d // P, f // P
    am2 = (anchor_2.astype(np.float64) ** 2).sum(1)
    c = 256.0 - 0.5 * am2
    ch = c.astype(np.float32).astype(ml_dtypes.bfloat16)
    cl = (c - ch.astype(np.float64)).astype(np.float32).astype(
        ml_dtypes.bfloat16)
    caug = np.zeros((4, m), dtype=ml_dtypes.bfloat16)
    caug[0] = ch
    caug[1] = cl
    shared = dict(
        anchT_h=anchor_2.T.astype(ml_dtypes.bfloat16),
        caug=caug,
        anchor=np.ascontiguousarray(anchor_2),
        wdim=(W_dim / K).astype(ml_dtypes.bfloat16),
        wfus=W_fus.astype(ml_dtypes.bfloat16),
        we1=W_e1.astype(ml_dtypes.bfloat16),
        we2=W_e2.astype(ml_dtypes.bfloat16),
        wd=W_d.astype(ml_dtypes.bfloat16),
        bdim=_chunk_vec(b_dim, DC), bfus=_chunk_vec(b_fus, DC),
        be1=_chunk_vec(b_e1, FC), be2=_chunk_vec(b_e2, DC),
        bd=_chunk_vec(b_d, DC),
        g1=_chunk_vec(g1, DC), bt1=_chunk_vec(bt1, DC),
        g2=_chunk_vec(g2, DC), bt2=_chunk_vec(bt2, DC),
        gd=_chunk_vec(g_d, DC), btd=_chunk_vec(bt_d, DC),
    )
    in_maps = []
    for cix in range(n_cores):
        shard = np.ascontiguousarray(src[cix * ns:(cix + 1) * ns])
        in_maps.append(dict(
            shared,
            srcT_h=shard.T.astype(ml_dtypes.bfloat16),
            src_nat=shard.astype(np.float32)))
    return in_maps


_NC_CACHE = {}


def kernel(**inputs):
    key = "full"
    if key not in _NC_CACHE:
        _NC_CACHE[key] = build_kernel()
    nc = _NC_CACHE[key]
    in_maps = prepare_inputs(**{k: np.asarray(v) for k, v in inputs.items()})
    res = run_bass_kernel_spmd(nc, in_maps, core_ids=list(range(N_CORES)))
    return np.concatenate([r["out"] for r in res.results], axis=0)
